# revision 1
# baseline (speedup 1.0000x reference)
"""GCN-GRU Trainium2 kernel.

Strategy
--------
The model is a 16384-step GRU recurrence over a 16-dim state with *per-step*
weight matrices (memory-bound: ~114 MB of per-step weights).  A literal serial
scan would pay per-instruction floors (~0.1-1 us) 16384 times.  Instead we use
the fact that the per-step map is strongly contractive (GRU gates ~0.5, small
weights): Jacobi/Picard iteration
    h^{k}[t] = F_t(h^{k-1}[t-1])   for all t in parallel
converges geometrically (~8x error reduction per sweep; float32-exact at 9
sweeps, verified empirically on the reference inputs; we run 8, giving
~2e-6 max abs error vs the reference scan).  Error from a frozen left
boundary decays per step of distance, so each of the 8 cores independently
processes its 2048-step slice plus a 128-step warm-up margin - zero
cross-core communication.

Per core:
  phase 0: build graph matrices B_m (I, Lsum, L_l @ Lsum) from a_list.
  phase 1: batched precompute over all t (t tiled 128/partition-dim):
     - effective hidden-GCN matrix  H~[t] = sum_m c_m(wh[t]) B_m  (one matmul
       per 128 steps), with gcn_bh folded in as a 17th column.
     - xg[t] = relu(sum_{c,m} cx_{c,m}(wx[t]) B_m x[t,:,c] + bx[t])
     - gate pre-activations U,V,W = xg @ K0/K2/K4 + biases
     - weight "streams" K13~[t] (h@K1|h@K3, bias row = U|V) and K5~[t]
       (bias row = W + B5), stored transposed so a batched mat-vec is a
       broadcast-multiply + grouped free-dim reduction on the Vector engine.
  phase 2: 12 Jacobi sweeps; each sweep = per-128-t-tile batched
     matvec/sigmoid/tanh (DVE + ACT), with one partition-shift DMA per sweep
     implementing h[t] <- h[t-1].
"""

import numpy as np
from contextlib import ExitStack

import concourse.bass as bass
import concourse.bacc as bacc
import concourse.tile as tile
from concourse import mybir
from concourse import masks
from concourse.bass_utils import run_bass_kernel_spmd

F32 = mybir.dt.float32
AF = mybir.ActivationFunctionType
OP = mybir.AluOpType
AX = mybir.AxisListType

P = 128          # timesteps per tile (partition dim)
N = 16           # graph nodes / state dim
S = N + 1        # state + bias/ones column
T_FULL = 16384
NCORES = 8
PER_CORE = T_FULL // NCORES   # 2048
MARGIN = 128                  # warm-up margin (multiple of P)
NTILES = (PER_CORE + MARGIN) // P   # 17
NSWEEP = 8
CHUNK = 6     # t-tiles fused per DVE instruction in phase 2


def _phase0(nc, pool, ps0, al_d):
    """Graph-structure matrices.  Returns (ident, Bflat_H [5,16,S],
    BflatT [16,5,16])."""
    # NOTE on staging copies: walrus's LDWEIGHTS lowering accepts only ONE
    # sync wait per Matmult, so every PE instruction's operands must have a
    # single-processor (DVE) dependency set.  DMA- or GPSIMD-produced tiles
    # are staged through a DVE tensor_copy before PE consumes them.
    ident_g = pool.tile([P, P], F32)
    masks.make_identity(nc, ident_g[:])
    ident = pool.tile([P, P], F32)
    nc.vector.tensor_copy(ident[:], ident_g[:])
    i16 = ident[0:16, 0:16]

    # a_rows[i, l, j] = a_list[l, i, j]
    a_rows_d = pool.tile([16, 3, 16], F32)
    nc.sync.dma_start(out=a_rows_d[:], in_=al_d.ap().transpose([1, 0, 2]))
    a_rows = pool.tile([16, 3, 16], F32)
    nc.vector.tensor_copy(a_rows[:], a_rows_d[:])

    ones16 = pool.tile([16, 1], F32)
    nc.vector.memset(ones16[:], 1.0)
    onesK = pool.tile([1, 16], F32)
    nc.vector.memset(onesK[:], 1.0)

    # column sums d[l, j] = sum_i a[l, i, j]  -> [48, 1] (partition = (l, j))
    d_ps = ps0.tile([48, 1], F32)
    nc.tensor.matmul(d_ps[:], a_rows[:].rearrange("i l j -> i (l j)"),
                     ones16[:], start=True, stop=True)
    d_sb = pool.tile([48, 1], F32)
    nc.vector.tensor_copy(d_sb[:], d_ps[:])

    # dis = 1/sqrt(d), with one Newton refinement (ACT Sqrt is low-precision)
    sq = pool.tile([48, 1], F32)
    nc.scalar.activation(sq[:], d_sb[:], AF.Sqrt)
    y0 = pool.tile([48, 1], F32)
    nc.vector.reciprocal(y0[:], sq[:])
    t1 = pool.tile([48, 1], F32)
    nc.vector.tensor_mul(t1[:], y0[:], y0[:])
    t2 = pool.tile([48, 1], F32)
    nc.vector.tensor_mul(t2[:], d_sb[:], t1[:])
    t3 = pool.tile([48, 1], F32)
    nc.vector.tensor_scalar(t3[:], t2[:], -0.5, 1.5, op0=OP.mult, op1=OP.add)
    dis = pool.tile([48, 1], F32)
    nc.vector.tensor_mul(dis[:], y0[:], t3[:])

    # reshape d / dis to [16 (partition=node), 3 (l)] via tiny SBUF->SBUF DMAs
    dP = pool.tile([16, 3], F32)
    disP = pool.tile([16, 3], F32)
    for l in range(3):
        nc.gpsimd.dma_start(out=dP[:, l:l + 1],
                            in_=d_sb[16 * l:16 * (l + 1), :])
        nc.gpsimd.dma_start(out=disP[:, l:l + 1],
                            in_=dis[16 * l:16 * (l + 1), :])
    # dis as a row, broadcast down 16 partitions via K=1 matmul
    disRow_d = pool.tile([1, 48], F32)
    nc.gpsimd.dma_start(out=disRow_d[:], in_=dis[:, :])
    disRow = pool.tile([1, 48], F32)
    nc.vector.tensor_copy(disRow[:], disRow_d[:])
    disF_ps = ps0.tile([16, 48], F32)
    nc.tensor.matmul(disF_ps[:], onesK[:], disRow[:], start=True, stop=True)
    disF = pool.tile([16, 3, 16], F32)
    nc.vector.tensor_copy(disF[:], disF_ps[:].rearrange("i (l j) -> i l j", l=3))

    # L_hat[l] = diag(dis_l) (diag(d_l) - A_l) diag(dis_l), rows on partitions
    Dt = pool.tile([16, 3, 16], F32)
    for l in range(3):
        nc.vector.tensor_scalar(Dt[:, l, :], i16, dP[:, l:l + 1], None,
                                op0=OP.mult)
    Lmat = pool.tile([16, 3, 16], F32)
    nc.vector.tensor_sub(Lmat[:], Dt[:], a_rows[:])
    Lr = pool.tile([16, 3, 16], F32)
    for l in range(3):
        nc.vector.tensor_scalar(Lr[:, l, :], Lmat[:, l, :], disP[:, l:l + 1],
                                None, op0=OP.mult)
    Lh = pool.tile([16, 3, 16], F32)
    nc.vector.tensor_mul(Lh[:], Lr[:], disF[:])

    # Lsum = sum_l L_hat[l]
    Lsum_a = pool.tile([16, 16], F32)
    nc.vector.tensor_add(Lsum_a[:], Lh[:, 0, :], Lh[:, 1, :])
    Lsum = pool.tile([16, 16], F32)
    nc.vector.tensor_add(Lsum[:], Lsum_a[:], Lh[:, 2, :])

    # transposes of L_hat[l]
    LhT = []
    for l in range(3):
        tp = ps0.tile([16, 16], F32, tag="tp")
        nc.tensor.transpose(tp[:], Lh[:, l, :], i16)
        lhT = pool.tile([16, 16], F32, tag=f"lhT{l}")
        nc.vector.tensor_copy(lhT[:], tp[:])
        LhT.append(lhT)
    LsumT_ps = ps0.tile([16, 16], F32, tag="tp")
    nc.tensor.transpose(LsumT_ps[:], Lsum[:], i16)
    LsumT = pool.tile([16, 16], F32)
    nc.vector.tensor_copy(LsumT[:], LsumT_ps[:])

    # BflatT[j, i, m] = B_m[i, j]  where B = (I, Lsum, L_hat[l] @ Lsum);
    # B^T_{2+l} = Lsum^T @ L_hat[l]^T.  (m innermost so the xg stage can
    # reduce over m with a grouped free-dim reduction.)
    BflatT = pool.tile([16, 16, 5], F32)
    nc.vector.tensor_copy(BflatT[:, :, 0], i16)
    nc.vector.tensor_copy(BflatT[:, :, 1], LsumT[:])
    for l in range(3):
        btps = ps0.tile([16, 16], F32, tag="bps")
        nc.tensor.matmul(btps[:], Lsum[:], LhT[l][:], start=True, stop=True)
        nc.vector.tensor_copy(BflatT[:, :, 2 + l], btps[:])

    # Row-major B matrices: B_{2+l} = L_hat[l] @ Lsum.
    Brows = pool.tile([16, 5, 16], F32)
    nc.vector.tensor_copy(Brows[:, 0, :], i16)
    nc.vector.tensor_copy(Brows[:, 1, :], Lsum[:])
    for l in range(3):
        bps = ps0.tile([16, 16], F32, tag="bps")
        nc.tensor.matmul(bps[:], LhT[l][:], Lsum[:], start=True, stop=True)
        nc.vector.tensor_copy(Brows[:, 2 + l, :], bps[:])

    # Bflat_H[m, i, j] = B_m[i, j] (j = S-1 column left zero for bias slot).
    # Move the m axis onto partitions with 16 per-j PE transposes of
    # Brows[:, :, j] ([16 i, 5 m] -> [5 m, 16 i]) instead of DMAs, so
    # consumers carry only PE/DVE semaphore waits (walrus caps sync waits
    # per instruction, and DMA-queue sems were blowing that cap).
    bh_ps = ps0.tile([5, 16, 16], F32)   # [m, j, i]
    for j in range(16):
        nc.tensor.transpose(bh_ps[:, j, :], Brows[:, :, j], i16)
    Bflat_H = pool.tile([5, 16, S], F32)
    nc.vector.memset(Bflat_H[:], 0.0)
    nc.vector.tensor_copy(Bflat_H[:, :, 0:16].transpose([0, 2, 1]), bh_ps[:])
    return ident, Bflat_H, BflatT


# packed small-input layout (host-side concat): wh | wx | x | bx | bh | gb
PK_W = 13 + 26 + 32 + 16 + 16 + 96   # 199


def _build(ntiles, nsweep):
    nt = ntiles * P
    nc = bacc.Bacc("TRN2", target_bir_lowering=False)
    pk_d = nc.dram_tensor("pk", [nt, PK_W], F32, kind="ExternalInput")
    gk_d = nc.dram_tensor("gk", [nt, 6, N, N], F32, kind="ExternalInput")
    al_d = nc.dram_tensor("alist", [3, N, N], F32, kind="ExternalInput")
    ho_d = nc.dram_tensor("hout", [nt, N], F32, kind="ExternalOutput")

    def body(ctx, tc):
        _body(ctx, tc, ntiles, nsweep, pk_d, gk_d, al_d, ho_d)

    with tile.TileContext(nc) as tc:
        with ExitStack() as ctx:
            body(ctx, tc)
    return nc


def _body(ctx, tc, ntiles, nsweep, pk_d, gk_d, al_d, ho_d):
    nc = tc.nc
    if True:
            const = ctx.enter_context(tc.tile_pool(name="const", bufs=1))
            with tc.tile_pool(name="ps0", bufs=1, space="PSUM") as ps0:
                ident, Bflat_H, BflatT = _phase0(nc, const, ps0, al_d)

            persist = ctx.enter_context(tc.tile_pool(name="persist", bufs=1))
            ld = ctx.enter_context(tc.tile_pool(name="ld", bufs=3))
            tmp = ctx.enter_context(tc.tile_pool(name="tmp", bufs=4))
            tmp2 = ctx.enter_context(tc.tile_pool(name="tmp2", bufs=2))
            psA = ctx.enter_context(tc.tile_pool(name="psA", bufs=2,
                                                 space="PSUM"))
            psB = ctx.enter_context(tc.tile_pool(name="psB", bufs=2,
                                                 space="PSUM"))

            # persistent streams + state
            Hs = persist.tile([P, ntiles, 16, S], F32)
            K13s = persist.tile([P, ntiles, 32, S], F32)
            K5s = persist.tile([P, ntiles, 16, S], F32)
            h_all = persist.tile([P, ntiles, 16], F32)
            # hprev is split per phase-2 chunk so a sweep's first chunks
            # don't wait on the last chunk's shift DMA (Tile tracks deps
            # at tile-object granularity).
            chunks = [(c0, min(c0 + CHUNK, ntiles))
                      for c0 in range(0, ntiles, CHUNK)]
            hprev_c = [persist.tile([P, c1 - c0, S], F32,
                                    name=f"hprev{c0}", tag=f"hprev{c0}")
                       for c0, c1 in chunks]
            hg_all = persist.tile([P, ntiles, S], F32)
            rh_all = persist.tile([P, ntiles, S], F32)
            hgpre = persist.tile([P, ntiles, 16], F32)
            rzpre = persist.tile([P, ntiles, 32], F32)
            hcpre = persist.tile([P, ntiles, 16], F32)
            rz_all = persist.tile([P, ntiles, 32], F32)
            hc_all = persist.tile([P, ntiles, 16], F32)

            nc.vector.memset(h_all[:], 0.0)
            nc.vector.memset(hg_all[:], 0.0)
            nc.vector.memset(rh_all[:], 0.0)
            nc.vector.memset(hg_all[:, :, 16], 1.0)
            nc.vector.memset(rh_all[:, :, 16], 1.0)
            for hp in hprev_c:
                nc.vector.memset(hp[:], 0.0)
                nc.vector.memset(hp[:, :, 16], 1.0)

            bh_rhs = Bflat_H[:].rearrange("m i j -> m (i j)")
            bt_rhs = BflatT[:].rearrange("j i m -> j (i m)")

            # ---------------- phase 1 ----------------
            for it in range(ntiles):
                t0 = it * P
                sl = slice(t0, t0 + P)
                pk_t = ld.tile([P, PK_W], F32)
                nc.sync.dma_start(out=pk_t[:], in_=pk_d[sl, :])
                gk_t = ld.tile([P, 6, N, N], F32)
                nc.sync.dma_start(out=gk_t[:], in_=gk_d[sl, :, :, :])
                wh_t = pk_t[:, 0:13]
                wx_t = pk_t[:, 13:39].rearrange("p (c k) -> p c k", c=2)
                x_t = pk_t[:, 39:71].rearrange("p (n c) -> p n c", n=N)
                bx_t = pk_t[:, 71:87]
                bh_t = pk_t[:, 87:103]
                gb_t = pk_t[:, 103:199].rearrange("p (k n) -> p k n", k=6)

                # (a) coefficients of H~ in the B_m basis:
                # c = (wh10, wh11*wh0, wh12*wh0*(wh0, wh1, wh2))
                csb = tmp.tile([P, 5], F32)
                nc.vector.tensor_copy(csb[:, 0:1], wh_t[:, 10:11])
                nc.vector.tensor_mul(csb[:, 1:2], wh_t[:, 11:12], wh_t[:, 0:1])
                t12 = tmp.tile([P, 1], F32)
                nc.vector.tensor_mul(t12[:], wh_t[:, 12:13], wh_t[:, 0:1])
                nc.vector.tensor_mul(csb[:, 2:5],
                                     t12[:].broadcast_to((P, 3)),
                                     wh_t[:, 0:3])
                ctp = psA.tile([5, P], F32, tag="ctp")
                nc.tensor.transpose(ctp[:], csb[:], ident[:])
                ctsb = tmp.tile([5, P], F32)
                nc.scalar.copy(ctsb[:], ctp[:])

                # (c) H~ tile: [P, 16*S] = cT^T @ Bflat_H
                hps = psB.tile([P, 16 * S], F32, tag="hps")
                nc.tensor.matmul(hps[:], ctsb[:], bh_rhs, start=True, stop=True)
                nc.scalar.copy(Hs[:, it],
                               hps[:].rearrange("p (i j) -> p i j", i=16))
                nc.vector.tensor_copy(Hs[:, it, :, 16], bh_t[:])

                # (d) xg   (x staged through DVE for the PE transposes)
                x2 = tmp.tile([P, N, 2], F32, tag="x2")
                nc.vector.tensor_copy(x2[:], x_t[:])
                xcts = []
                for c in range(2):
                    xps = psA.tile([16, P], F32, tag="xps")
                    nc.tensor.transpose(xps[:], x2[:, :, c], ident[:])
                    xct = tmp.tile([16, P], F32, tag=f"xct{c}")
                    nc.scalar.copy(xct[:], xps[:])
                    xcts.append(xct)
                yps = psB.tile([P, 2, 16, 5], F32, tag="yps")
                for c in range(2):
                    nc.tensor.matmul(
                        yps[:, c].rearrange("p i m -> p (i m)"), xcts[c][:],
                        bt_rhs, start=True, stop=True)
                cx = tmp.tile([P, 2, 5], F32)
                for c in range(2):
                    w0 = wx_t[:, c, 0:1]
                    nc.vector.tensor_copy(cx[:, c, 0:1], wx_t[:, c, 10:11])
                    nc.vector.tensor_mul(cx[:, c, 1:2], wx_t[:, c, 11:12], w0)
                    tc2 = tmp.tile([P, 1], F32, tag="tc2")
                    nc.vector.tensor_mul(tc2[:], wx_t[:, c, 12:13], w0)
                    nc.vector.tensor_mul(cx[:, c, 2:5],
                                         tc2[:].broadcast_to((P, 3)),
                                         wx_t[:, c, 0:3])
                # xg = relu(sum_{c,m} cx[c,m] * Y[c,:,m] + bx)
                t160 = tmp.tile([P, 2, 16, 5], F32, tag="t160")
                nc.vector.tensor_mul(
                    t160[:], yps[:],
                    cx[:].unsqueeze(2).broadcast_to((P, 2, 16, 5)))
                xsum = tmp.tile([P, 2, 16], F32, tag="xsum")
                nc.vector.tensor_reduce(xsum[:], t160[:], axis=AX.X, op=OP.add)
                xacc = tmp.tile([P, 16], F32, tag="accA")
                nc.vector.tensor_add(xacc[:], xsum[:, 0, :], xsum[:, 1, :])
                xacc2 = tmp.tile([P, 16], F32, tag="accB")
                nc.vector.tensor_add(xacc2[:], xacc[:], bx_t[:])
                xgt = tmp.tile([P, 16], F32, tag="xgt")
                nc.scalar.activation(xgt[:], xacc2[:], AF.Relu)

                # (e) U|V|W = xg @ K0|K2|K4, reading gk_t directly with a
                # (k, q outer, i inner) transposed view; biases folded below.
                UVW = tmp.tile([P, 3, 16], F32, tag="UVW")
                tqi = tmp.tile([P, 3, 16, 16], F32, tag="tqi")
                nc.vector.tensor_mul(
                    tqi[:], gk_t[:, 0:5:2].transpose([0, 1, 3, 2]),
                    xgt[:].unsqueeze(1).unsqueeze(1).broadcast_to(
                        (P, 3, 16, 16)))
                nc.vector.tensor_reduce(UVW[:], tqi[:], axis=AX.X, op=OP.add)
                UVW = UVW[:].rearrange("p a b -> p (a b)")

                # (f) phase-2 streams (bias rows j=16 carry U+B0+B1 | V+B2+B3
                # and W+B4+B5)
                for idx, k in enumerate((1, 3)):
                    nc.scalar.copy(
                        K13s[:, it, idx * 16:(idx + 1) * 16, 0:16],
                        gk_t[:, k].transpose([0, 2, 1]))
                gbs = tmp.tile([P, 3, 16], F32, tag="gbs")
                nc.vector.tensor_add(gbs[:, 0, :], gb_t[:, 0], gb_t[:, 1])
                nc.vector.tensor_add(gbs[:, 1, :], gb_t[:, 2], gb_t[:, 3])
                nc.vector.tensor_add(gbs[:, 2, :], gb_t[:, 4], gb_t[:, 5])
                nc.vector.tensor_add(K13s[:, it, :, 16], UVW[:, 0:32],
                                     gbs[:].rearrange("p a b -> p (a b)")[:, 0:32])
                nc.scalar.copy(K5s[:, it, :, 0:16],
                               gk_t[:, 5].transpose([0, 2, 1]))
                nc.vector.tensor_add(K5s[:, it, :, 16], UVW[:, 32:48],
                                     gbs[:, 2, :])

            # ---------------- phase 2: Jacobi sweeps ----------------
            for s in range(nsweep):
                for ci, (c0, c1) in enumerate(chunks):
                    cw = c1 - c0
                    t272 = tmp2.tile([P, CHUNK, 16, S], F32, tag="t272")
                    nc.vector.tensor_mul(
                        t272[:, :cw], Hs[:, c0:c1],
                        hprev_c[ci][:].unsqueeze(2).broadcast_to(
                            (P, cw, 16, S)))
                    nc.vector.tensor_reduce(hgpre[:, c0:c1], t272[:, :cw],
                                            axis=AX.X, op=OP.add)
                for c0, c1 in chunks:
                    nc.scalar.activation(hg_all[:, c0:c1, 0:16],
                                         hgpre[:, c0:c1], AF.Relu)
                for c0, c1 in chunks:
                    cw = c1 - c0
                    t544 = tmp2.tile([P, CHUNK, 32, S], F32, tag="t544")
                    nc.vector.tensor_mul(
                        t544[:, :cw], K13s[:, c0:c1],
                        hg_all[:, c0:c1].unsqueeze(2).broadcast_to(
                            (P, cw, 32, S)))
                    nc.vector.tensor_reduce(rzpre[:, c0:c1], t544[:, :cw],
                                            axis=AX.X, op=OP.add)
                for c0, c1 in chunks:
                    nc.scalar.activation(rz_all[:, c0:c1], rzpre[:, c0:c1],
                                         AF.Sigmoid)
                for c0, c1 in chunks:
                    nc.vector.tensor_mul(rh_all[:, c0:c1, 0:16],
                                         rz_all[:, c0:c1, 0:16],
                                         hg_all[:, c0:c1, 0:16])
                for c0, c1 in chunks:
                    cw = c1 - c0
                    t272b = tmp2.tile([P, CHUNK, 16, S], F32, tag="t272")
                    nc.vector.tensor_mul(
                        t272b[:, :cw], K5s[:, c0:c1],
                        rh_all[:, c0:c1].unsqueeze(2).broadcast_to(
                            (P, cw, 16, S)))
                    nc.vector.tensor_reduce(hcpre[:, c0:c1], t272b[:, :cw],
                                            axis=AX.X, op=OP.add)
                for c0, c1 in chunks:
                    nc.scalar.activation(hc_all[:, c0:c1], hcpre[:, c0:c1],
                                         AF.Tanh)
                for ci, (c0, c1) in enumerate(chunks):
                    cw = c1 - c0
                    dd = tmp2.tile([P, CHUNK, 16], F32, tag="dd")
                    nc.vector.tensor_sub(dd[:, :cw], hg_all[:, c0:c1, 0:16],
                                         hc_all[:, c0:c1])
                    ee = tmp2.tile([P, CHUNK, 16], F32, tag="ee")
                    nc.vector.tensor_mul(ee[:, :cw], rz_all[:, c0:c1, 16:32],
                                         dd[:, :cw])
                    nc.vector.tensor_add(h_all[:, c0:c1], hc_all[:, c0:c1],
                                         ee[:, :cw])
                    if s < nsweep - 1:
                        # incremental shift for the next sweep, overlapped
                        # with the remaining chunks' compute:
                        # hprev[p, t, :] <- h_all[p-1, t, :] within the tile,
                        # the p=0 row from partition 127 of tile t-1, and
                        # the next chunk's first p=0 row (tile 0 row 0 stays
                        # frozen at zero).
                        hp = hprev_c[ci]
                        nc.sync.dma_start(out=hp[1:P, :, 0:16],
                                          in_=h_all[0:P - 1, c0:c1, :])
                        if cw > 1:
                            nc.sync.dma_start(
                                out=hp[0:1, 1:cw, 0:16],
                                in_=h_all[P - 1:P, c0:c1 - 1, :])
                        if ci + 1 < len(chunks):
                            nc.sync.dma_start(
                                out=hprev_c[ci + 1][0:1, 0:1, 0:16],
                                in_=h_all[P - 1:P, c1 - 1:c1, :])

            # ---------------- output ----------------
            nc.sync.dma_start(
                out=ho_d.ap().rearrange("(a p) n -> p a n", p=P),
                in_=h_all[:])


def _pad_slice(a, lo, hi):
    """a[lo:hi] with zero-padding for lo < 0."""
    if lo >= 0:
        return np.ascontiguousarray(a[lo:hi])
    pad = np.zeros((-lo,) + a.shape[1:], a.dtype)
    return np.ascontiguousarray(np.concatenate([pad, a[0:hi]], axis=0))


def pack_small(wh, wx, x, bx, bh, gb):
    """Concatenate the small per-timestep inputs into one [T, 199] array
    (layout consumed by the kernel's pk tensor)."""
    n = wh.shape[0]
    return np.ascontiguousarray(np.concatenate([
        wh.reshape(n, -1), wx.reshape(n, -1), x.reshape(n, -1),
        bx.reshape(n, -1), bh.reshape(n, -1), gb.reshape(n, -1)],
        axis=1).astype(np.float32))


def kernel(inputs, a_list, gcn_wx, gcn_bx, gcn_wh, gcn_bh, gru_k, gru_b):
    inputs = np.ascontiguousarray(np.asarray(inputs, np.float32))
    a_list = np.ascontiguousarray(np.asarray(a_list, np.float32))
    gcn_wx = np.ascontiguousarray(np.asarray(gcn_wx, np.float32))
    gcn_bx = np.ascontiguousarray(np.asarray(gcn_bx, np.float32))
    gcn_wh = np.ascontiguousarray(np.asarray(gcn_wh, np.float32))
    gcn_bh = np.ascontiguousarray(np.asarray(gcn_bh, np.float32))
    gru_k = np.ascontiguousarray(np.asarray(gru_k, np.float32))
    gru_b = np.ascontiguousarray(np.asarray(gru_b, np.float32))

    nc = _build(NTILES, NSWEEP)
    if not nc.is_finalized():
        nc.finalize()

    in_maps = []
    for c in range(NCORES):
        lo = c * PER_CORE - MARGIN
        hi = c * PER_CORE + PER_CORE
        in_maps.append({
            "pk": pack_small(_pad_slice(gcn_wh, lo, hi)[:, 0, :],
                             _pad_slice(gcn_wx, lo, hi),
                             _pad_slice(inputs, lo, hi),
                             _pad_slice(gcn_bx, lo, hi),
                             _pad_slice(gcn_bh, lo, hi),
                             _pad_slice(gru_b, lo, hi)),
            "gk": _pad_slice(gru_k, lo, hi),
            "alist": a_list,
        })

    res = run_bass_kernel_spmd(nc, in_maps, core_ids=list(range(NCORES)))
    global LAST_RESULTS
    LAST_RESULTS = res
    out = np.concatenate(
        [res.results[c]["hout"][MARGIN:] for c in range(NCORES)], axis=0)
    return out.astype(np.float32)


LAST_RESULTS = None



# revision 6
# speedup vs baseline: 2.3738x; 2.3738x over previous
"""GCN-GRU Trainium2 kernel.

Strategy
--------
The model is a 16384-step GRU recurrence over a 16-dim state with *per-step*
weight matrices (memory-bound: ~114 MB of per-step weights).  A literal serial
scan would pay per-instruction floors (~0.1-1 us) 16384 times.  Instead we use
the fact that the per-step map is strongly contractive (GRU gates ~0.5, small
weights): Jacobi/Picard iteration
    h^{k}[t] = F_t(h^{k-1}[t-1])   for all t in parallel
converges geometrically (~8x error reduction per sweep; float32-exact at 9
sweeps, verified empirically on the reference inputs; we run 8, giving
~2e-6 max abs error vs the reference scan).  Error from a frozen left
boundary decays per step of distance, so each of the 8 cores independently
processes its 2048-step slice plus a 128-step warm-up margin - zero
cross-core communication.

Per core:
  phase 0: build graph matrices B_m (I, Lsum, L_l @ Lsum) from a_list.
  phase 1: batched precompute over all t (t tiled 128/partition-dim):
     - effective hidden-GCN matrix  H~[t] = sum_m c_m(wh[t]) B_m  (one matmul
       per 128 steps), with gcn_bh folded in as a 17th column.
     - xg[t] = relu(sum_{c,m} cx_{c,m}(wx[t]) B_m x[t,:,c] + bx[t])
     - gate pre-activations U,V,W = xg @ K0/K2/K4 + biases
     - weight "streams" K13~[t] (h@K1|h@K3, bias row = U|V) and K5~[t]
       (bias row = W + B5), stored transposed so a batched mat-vec is a
       broadcast-multiply + grouped free-dim reduction on the Vector engine.
  phase 2: 12 Jacobi sweeps; each sweep = per-128-t-tile batched
     matvec/sigmoid/tanh (DVE + ACT), with one partition-shift DMA per sweep
     implementing h[t] <- h[t-1].
"""

import numpy as np
from contextlib import ExitStack

import concourse.bass as bass
import concourse.bacc as bacc
import concourse.tile as tile
from concourse import mybir
from concourse import masks
from concourse.bass_utils import run_bass_kernel_spmd

F32 = mybir.dt.float32
F16 = mybir.dt.float16
AF = mybir.ActivationFunctionType
OP = mybir.AluOpType
AX = mybir.AxisListType

P = 128          # timesteps per tile (partition dim)
N = 16           # graph nodes / state dim
S = N + 1        # state + bias/ones column
T_FULL = 16384
NCORES = 8
PER_CORE = T_FULL // NCORES   # 2048
MARGIN = 128                  # warm-up margin (multiple of P)
NTILES = (PER_CORE + MARGIN) // P   # 17
NSWEEP = 8
CHUNK = 6     # t-tiles fused per DVE instruction in phase 2


def _phase0(nc, pool, ps0, al_d):
    """Graph-structure matrices.  Returns (ident, Bflat_H [5,16,S],
    BflatT [16,5,16])."""
    # NOTE on staging copies: walrus's LDWEIGHTS lowering accepts only ONE
    # sync wait per Matmult, so every PE instruction's operands must have a
    # single-processor (DVE) dependency set.  DMA- or GPSIMD-produced tiles
    # are staged through a DVE tensor_copy before PE consumes them.
    ident_g = pool.tile([P, P], F32)
    masks.make_identity(nc, ident_g[:])
    ident = pool.tile([P, P], F32)
    nc.vector.tensor_copy(ident[:], ident_g[:])
    i16 = ident[0:16, 0:16]

    # a_rows[i, l, j] = a_list[l, i, j]
    a_rows_d = pool.tile([16, 3, 16], F32)
    nc.sync.dma_start(out=a_rows_d[:], in_=al_d.ap().transpose([1, 0, 2]))
    a_rows = pool.tile([16, 3, 16], F32)
    nc.vector.tensor_copy(a_rows[:], a_rows_d[:])

    ones16 = pool.tile([16, 1], F32)
    nc.vector.memset(ones16[:], 1.0)
    onesK = pool.tile([1, 16], F32)
    nc.vector.memset(onesK[:], 1.0)

    # column sums d[l, j] = sum_i a[l, i, j]  -> [48, 1] (partition = (l, j))
    d_ps = ps0.tile([48, 1], F32)
    nc.tensor.matmul(d_ps[:], a_rows[:].rearrange("i l j -> i (l j)"),
                     ones16[:], start=True, stop=True)
    d_sb = pool.tile([48, 1], F32)
    nc.vector.tensor_copy(d_sb[:], d_ps[:])

    # dis = 1/sqrt(d), with one Newton refinement (ACT Sqrt is low-precision)
    sq = pool.tile([48, 1], F32)
    nc.scalar.activation(sq[:], d_sb[:], AF.Sqrt)
    y0 = pool.tile([48, 1], F32)
    nc.vector.reciprocal(y0[:], sq[:])
    t1 = pool.tile([48, 1], F32)
    nc.vector.tensor_mul(t1[:], y0[:], y0[:])
    t2 = pool.tile([48, 1], F32)
    nc.vector.tensor_mul(t2[:], d_sb[:], t1[:])
    t3 = pool.tile([48, 1], F32)
    nc.vector.tensor_scalar(t3[:], t2[:], -0.5, 1.5, op0=OP.mult, op1=OP.add)
    dis = pool.tile([48, 1], F32)
    nc.vector.tensor_mul(dis[:], y0[:], t3[:])

    # reshape d / dis to [16 (partition=node), 3 (l)] via tiny SBUF->SBUF DMAs
    dP = pool.tile([16, 3], F32)
    disP = pool.tile([16, 3], F32)
    for l in range(3):
        nc.gpsimd.dma_start(out=dP[:, l:l + 1],
                            in_=d_sb[16 * l:16 * (l + 1), :])
        nc.gpsimd.dma_start(out=disP[:, l:l + 1],
                            in_=dis[16 * l:16 * (l + 1), :])
    # dis as a row, broadcast down 16 partitions via K=1 matmul
    disRow_d = pool.tile([1, 48], F32)
    nc.gpsimd.dma_start(out=disRow_d[:], in_=dis[:, :])
    disRow = pool.tile([1, 48], F32)
    nc.vector.tensor_copy(disRow[:], disRow_d[:])
    disF_ps = ps0.tile([16, 48], F32)
    nc.tensor.matmul(disF_ps[:], onesK[:], disRow[:], start=True, stop=True)
    disF = pool.tile([16, 3, 16], F32)
    nc.vector.tensor_copy(disF[:], disF_ps[:].rearrange("i (l j) -> i l j", l=3))

    # L_hat[l] = diag(dis_l) (diag(d_l) - A_l) diag(dis_l), rows on partitions
    Dt = pool.tile([16, 3, 16], F32)
    for l in range(3):
        nc.vector.tensor_scalar(Dt[:, l, :], i16, dP[:, l:l + 1], None,
                                op0=OP.mult)
    Lmat = pool.tile([16, 3, 16], F32)
    nc.vector.tensor_sub(Lmat[:], Dt[:], a_rows[:])
    Lr = pool.tile([16, 3, 16], F32)
    for l in range(3):
        nc.vector.tensor_scalar(Lr[:, l, :], Lmat[:, l, :], disP[:, l:l + 1],
                                None, op0=OP.mult)
    Lh = pool.tile([16, 3, 16], F32)
    nc.vector.tensor_mul(Lh[:], Lr[:], disF[:])

    # Lsum = sum_l L_hat[l]
    Lsum_a = pool.tile([16, 16], F32)
    nc.vector.tensor_add(Lsum_a[:], Lh[:, 0, :], Lh[:, 1, :])
    Lsum = pool.tile([16, 16], F32)
    nc.vector.tensor_add(Lsum[:], Lsum_a[:], Lh[:, 2, :])

    # transposes of L_hat[l]
    LhT = []
    for l in range(3):
        tp = ps0.tile([16, 16], F32, tag="tp")
        nc.tensor.transpose(tp[:], Lh[:, l, :], i16)
        lhT = pool.tile([16, 16], F32, tag=f"lhT{l}")
        nc.vector.tensor_copy(lhT[:], tp[:])
        LhT.append(lhT)
    LsumT_ps = ps0.tile([16, 16], F32, tag="tp")
    nc.tensor.transpose(LsumT_ps[:], Lsum[:], i16)
    LsumT = pool.tile([16, 16], F32)
    nc.vector.tensor_copy(LsumT[:], LsumT_ps[:])

    # BflatT[j, i, m] = B_m[i, j]  where B = (I, Lsum, L_hat[l] @ Lsum);
    # B^T_{2+l} = Lsum^T @ L_hat[l]^T.  (m innermost so the xg stage can
    # reduce over m with a grouped free-dim reduction.)
    BflatT = pool.tile([16, 16, 5], F32)
    nc.vector.tensor_copy(BflatT[:, :, 0], i16)
    nc.vector.tensor_copy(BflatT[:, :, 1], LsumT[:])
    for l in range(3):
        btps = ps0.tile([16, 16], F32, tag="bps")
        nc.tensor.matmul(btps[:], Lsum[:], LhT[l][:], start=True, stop=True)
        nc.vector.tensor_copy(BflatT[:, :, 2 + l], btps[:])

    # Row-major B matrices: B_{2+l} = L_hat[l] @ Lsum.
    Brows = pool.tile([16, 5, 16], F32)
    nc.vector.tensor_copy(Brows[:, 0, :], i16)
    nc.vector.tensor_copy(Brows[:, 1, :], Lsum[:])
    for l in range(3):
        bps = ps0.tile([16, 16], F32, tag="bps")
        nc.tensor.matmul(bps[:], LhT[l][:], Lsum[:], start=True, stop=True)
        nc.vector.tensor_copy(Brows[:, 2 + l, :], bps[:])

    # Bflat_H[m, i, j] = B_m[i, j] (j = S-1 column left zero for bias slot).
    # Move the m axis onto partitions with 16 per-j PE transposes of
    # Brows[:, :, j] ([16 i, 5 m] -> [5 m, 16 i]) instead of DMAs, so
    # consumers carry only PE/DVE semaphore waits (walrus caps sync waits
    # per instruction, and DMA-queue sems were blowing that cap).
    bh_ps = ps0.tile([5, 16, 16], F32)   # [m, j, i]
    for j in range(16):
        nc.tensor.transpose(bh_ps[:, j, :], Brows[:, :, j], i16)
    Bflat_H = pool.tile([5, 16, S], F32)
    nc.vector.memset(Bflat_H[:], 0.0)
    nc.vector.tensor_copy(Bflat_H[:, :, 0:16].transpose([0, 2, 1]), bh_ps[:])
    return ident, Bflat_H, BflatT


# packed small-input layout (host-side concat): wh | wx | x | bx | bh | gb
PK_W = 13 + 26 + 32 + 16 + 16 + 96   # 199


def _build(ntiles, nsweep):
    nt = ntiles * P
    nc = bacc.Bacc("TRN2", target_bir_lowering=False)
    pk_d = nc.dram_tensor("pk", [nt, PK_W], F16, kind="ExternalInput")
    gk_d = nc.dram_tensor("gk", [nt, 6, N, N], F16, kind="ExternalInput")
    al_d = nc.dram_tensor("alist", [3, N, N], F32, kind="ExternalInput")
    ho_d = nc.dram_tensor("hout", [nt, N], F32, kind="ExternalOutput")

    def body(ctx, tc):
        _body(ctx, tc, ntiles, nsweep, pk_d, gk_d, al_d, ho_d)

    with tile.TileContext(nc) as tc:
        with ExitStack() as ctx:
            body(ctx, tc)
    return nc


def _body(ctx, tc, ntiles, nsweep, pk_d, gk_d, al_d, ho_d):
    nc = tc.nc
    if True:
            const = ctx.enter_context(tc.tile_pool(name="const", bufs=1))
            with tc.tile_pool(name="ps0", bufs=1, space="PSUM") as ps0:
                ident, Bflat_H, BflatT = _phase0(nc, const, ps0, al_d)

            persist = ctx.enter_context(tc.tile_pool(name="persist", bufs=1))
            ld = ctx.enter_context(tc.tile_pool(name="ld", bufs=3))
            tmp = ctx.enter_context(tc.tile_pool(name="tmp", bufs=4))
            tmp2 = ctx.enter_context(tc.tile_pool(name="tmp2", bufs=2))
            psA = ctx.enter_context(tc.tile_pool(name="psA", bufs=2,
                                                 space="PSUM"))
            psB = ctx.enter_context(tc.tile_pool(name="psB", bufs=2,
                                                 space="PSUM"))

            # persistent streams + state
            Hs = persist.tile([P, ntiles, 16, S], F32)
            K13s = persist.tile([P, ntiles, 32, S], F32)
            K5s = persist.tile([P, ntiles, 16, S], F32)
            h_all = persist.tile([P, ntiles, 16], F32)
            # hprev is split per phase-2 chunk so a sweep's first chunks
            # don't wait on the last chunk's shift DMA (Tile tracks deps
            # at tile-object granularity).
            chunks = [(c0, min(c0 + CHUNK, ntiles))
                      for c0 in range(0, ntiles, CHUNK)]
            hprev_c = [persist.tile([P, c1 - c0, S], F32,
                                    name=f"hprev{c0}", tag=f"hprev{c0}")
                       for c0, c1 in chunks]
            hg_all = persist.tile([P, ntiles, S], F32)
            rh_all = persist.tile([P, ntiles, S], F32)
            hgpre = persist.tile([P, ntiles, 16], F32)
            rzpre = persist.tile([P, ntiles, 32], F32)
            hcpre = persist.tile([P, ntiles, 16], F32)
            rz_all = persist.tile([P, ntiles, 32], F32)
            hc_all = persist.tile([P, ntiles, 16], F32)

            nc.vector.memset(h_all[:], 0.0)
            nc.vector.memset(hg_all[:], 0.0)
            nc.vector.memset(rh_all[:], 0.0)
            nc.vector.memset(hg_all[:, :, 16], 1.0)
            nc.vector.memset(rh_all[:, :, 16], 1.0)
            for hp in hprev_c:
                nc.vector.memset(hp[:], 0.0)
                nc.vector.memset(hp[:, :, 16], 1.0)

            bh_rhs = Bflat_H[:].rearrange("m i j -> m (i j)")
            bt_rhs = BflatT[:].rearrange("j i m -> j (i m)")

            # ---------------- phase 1 ----------------
            for it in range(ntiles):
                t0 = it * P
                sl = slice(t0, t0 + P)
                pk_h = ld.tile([P, PK_W], F16, tag="pk_h")
                nc.sync.dma_start(out=pk_h[:], in_=pk_d[sl, :])
                gk_h = ld.tile([P, 6, N, N], F16, tag="gk_h")
                nc.sync.dma_start(out=gk_h[:], in_=gk_d[sl, :, :, :])
                pk_t = ld.tile([P, PK_W], F32, tag="pk_t")
                nc.vector.tensor_copy(pk_t[:], pk_h[:])
                gk_t = ld.tile([P, 6, N, N], F32, tag="gk_t")
                nc.vector.tensor_copy(gk_t[:], gk_h[:])
                wh_t = pk_t[:, 0:13]
                wx_t = pk_t[:, 13:39].rearrange("p (c k) -> p c k", c=2)
                x_t = pk_t[:, 39:71].rearrange("p (n c) -> p n c", n=N)
                bx_t = pk_t[:, 71:87]
                bh_t = pk_t[:, 87:103]
                gb_t = pk_t[:, 103:199].rearrange("p (k n) -> p k n", k=6)

                # (a) coefficients of H~ in the B_m basis:
                # c = (wh10, wh11*wh0, wh12*wh0*(wh0, wh1, wh2))
                csb = tmp.tile([P, 5], F32)
                nc.vector.tensor_copy(csb[:, 0:1], wh_t[:, 10:11])
                nc.vector.tensor_mul(csb[:, 1:2], wh_t[:, 11:12], wh_t[:, 0:1])
                t12 = tmp.tile([P, 1], F32)
                nc.vector.tensor_mul(t12[:], wh_t[:, 12:13], wh_t[:, 0:1])
                nc.vector.tensor_mul(csb[:, 2:5],
                                     t12[:].broadcast_to((P, 3)),
                                     wh_t[:, 0:3])
                ctp = psA.tile([5, P], F32, tag="ctp")
                nc.tensor.transpose(ctp[:], csb[:], ident[:])
                ctsb = tmp.tile([5, P], F32)
                nc.scalar.copy(ctsb[:], ctp[:])

                # (c) H~ tile: [P, 16*S] = cT^T @ Bflat_H
                hps = psB.tile([P, 16 * S], F32, tag="hps")
                nc.tensor.matmul(hps[:], ctsb[:], bh_rhs, start=True, stop=True)
                nc.scalar.copy(Hs[:, it],
                               hps[:].rearrange("p (i j) -> p i j", i=16))
                nc.vector.tensor_copy(Hs[:, it, :, 16], bh_t[:])

                # (d) xg   (x staged through DVE for the PE transposes)
                x2 = tmp.tile([P, N, 2], F32, tag="x2")
                nc.vector.tensor_copy(x2[:], x_t[:])
                xcts = []
                for c in range(2):
                    xps = psA.tile([16, P], F32, tag="xps")
                    nc.tensor.transpose(xps[:], x2[:, :, c], ident[:])
                    xct = tmp.tile([16, P], F32, tag=f"xct{c}")
                    nc.scalar.copy(xct[:], xps[:])
                    xcts.append(xct)
                yps = psB.tile([P, 2, 16, 5], F32, tag="yps")
                for c in range(2):
                    nc.tensor.matmul(
                        yps[:, c].rearrange("p i m -> p (i m)"), xcts[c][:],
                        bt_rhs, start=True, stop=True)
                cx = tmp.tile([P, 2, 5], F32)
                for c in range(2):
                    w0 = wx_t[:, c, 0:1]
                    nc.vector.tensor_copy(cx[:, c, 0:1], wx_t[:, c, 10:11])
                    nc.vector.tensor_mul(cx[:, c, 1:2], wx_t[:, c, 11:12], w0)
                    tc2 = tmp.tile([P, 1], F32, tag="tc2")
                    nc.vector.tensor_mul(tc2[:], wx_t[:, c, 12:13], w0)
                    nc.vector.tensor_mul(cx[:, c, 2:5],
                                         tc2[:].broadcast_to((P, 3)),
                                         wx_t[:, c, 0:3])
                # xg = relu(sum_{c,m} cx[c,m] * Y[c,:,m] + bx)
                t160 = tmp.tile([P, 2, 16, 5], F32, tag="t160")
                nc.vector.tensor_mul(
                    t160[:], yps[:],
                    cx[:].unsqueeze(2).broadcast_to((P, 2, 16, 5)))
                xsum = tmp.tile([P, 2, 16], F32, tag="xsum")
                nc.vector.tensor_reduce(xsum[:], t160[:], axis=AX.X, op=OP.add)
                xacc = tmp.tile([P, 16], F32, tag="accA")
                nc.vector.tensor_add(xacc[:], xsum[:, 0, :], xsum[:, 1, :])
                xacc2 = tmp.tile([P, 16], F32, tag="accB")
                nc.vector.tensor_add(xacc2[:], xacc[:], bx_t[:])
                xgt = tmp.tile([P, 16], F32, tag="xgt")
                nc.scalar.activation(xgt[:], xacc2[:], AF.Relu)

                # (e) U|V|W = xg @ K0|K2|K4, reading gk_t directly with a
                # (k, q outer, i inner) transposed view; biases folded below.
                UVW = tmp.tile([P, 3, 16], F32, tag="UVW")
                tqi = tmp.tile([P, 3, 16, 16], F32, tag="tqi")
                nc.vector.tensor_mul(
                    tqi[:], gk_t[:, 0:5:2].transpose([0, 1, 3, 2]),
                    xgt[:].unsqueeze(1).unsqueeze(1).broadcast_to(
                        (P, 3, 16, 16)))
                nc.vector.tensor_reduce(UVW[:], tqi[:], axis=AX.X, op=OP.add)
                UVW = UVW[:].rearrange("p a b -> p (a b)")

                # (f) phase-2 streams (bias rows j=16 carry U+B0+B1 | V+B2+B3
                # and W+B4+B5)
                for idx, k in enumerate((1, 3)):
                    nc.scalar.copy(
                        K13s[:, it, idx * 16:(idx + 1) * 16, 0:16],
                        gk_t[:, k].transpose([0, 2, 1]))
                gbs = tmp.tile([P, 3, 16], F32, tag="gbs")
                nc.vector.tensor_add(gbs[:, 0, :], gb_t[:, 0], gb_t[:, 1])
                nc.vector.tensor_add(gbs[:, 1, :], gb_t[:, 2], gb_t[:, 3])
                nc.vector.tensor_add(gbs[:, 2, :], gb_t[:, 4], gb_t[:, 5])
                nc.vector.tensor_add(K13s[:, it, :, 16], UVW[:, 0:32],
                                     gbs[:].rearrange("p a b -> p (a b)")[:, 0:32])
                nc.scalar.copy(K5s[:, it, :, 0:16],
                               gk_t[:, 5].transpose([0, 2, 1]))
                nc.vector.tensor_add(K5s[:, it, :, 16], UVW[:, 32:48],
                                     gbs[:, 2, :])

            # ---------------- phase 2: Jacobi sweeps ----------------
            for s in range(nsweep):
                for ci, (c0, c1) in enumerate(chunks):
                    cw = c1 - c0
                    t272 = tmp2.tile([P, CHUNK, 16, S], F32, tag="t272")
                    nc.vector.tensor_mul(
                        t272[:, :cw], Hs[:, c0:c1],
                        hprev_c[ci][:].unsqueeze(2).broadcast_to(
                            (P, cw, 16, S)))
                    nc.vector.tensor_reduce(hgpre[:, c0:c1], t272[:, :cw],
                                            axis=AX.X, op=OP.add)
                for c0, c1 in chunks:
                    nc.scalar.activation(hg_all[:, c0:c1, 0:16],
                                         hgpre[:, c0:c1], AF.Relu)
                for c0, c1 in chunks:
                    cw = c1 - c0
                    t544 = tmp2.tile([P, CHUNK, 32, S], F32, tag="t544")
                    nc.vector.tensor_mul(
                        t544[:, :cw], K13s[:, c0:c1],
                        hg_all[:, c0:c1].unsqueeze(2).broadcast_to(
                            (P, cw, 32, S)))
                    nc.vector.tensor_reduce(rzpre[:, c0:c1], t544[:, :cw],
                                            axis=AX.X, op=OP.add)
                for c0, c1 in chunks:
                    nc.scalar.activation(rz_all[:, c0:c1], rzpre[:, c0:c1],
                                         AF.Sigmoid)
                for c0, c1 in chunks:
                    nc.vector.tensor_mul(rh_all[:, c0:c1, 0:16],
                                         rz_all[:, c0:c1, 0:16],
                                         hg_all[:, c0:c1, 0:16])
                for c0, c1 in chunks:
                    cw = c1 - c0
                    t272b = tmp2.tile([P, CHUNK, 16, S], F32, tag="t272")
                    nc.vector.tensor_mul(
                        t272b[:, :cw], K5s[:, c0:c1],
                        rh_all[:, c0:c1].unsqueeze(2).broadcast_to(
                            (P, cw, 16, S)))
                    nc.vector.tensor_reduce(hcpre[:, c0:c1], t272b[:, :cw],
                                            axis=AX.X, op=OP.add)
                for c0, c1 in chunks:
                    nc.scalar.activation(hc_all[:, c0:c1], hcpre[:, c0:c1],
                                         AF.Tanh)
                for ci, (c0, c1) in enumerate(chunks):
                    cw = c1 - c0
                    dd = tmp2.tile([P, CHUNK, 16], F32, tag="dd")
                    nc.vector.tensor_sub(dd[:, :cw], hg_all[:, c0:c1, 0:16],
                                         hc_all[:, c0:c1])
                    ee = tmp2.tile([P, CHUNK, 16], F32, tag="ee")
                    nc.vector.tensor_mul(ee[:, :cw], rz_all[:, c0:c1, 16:32],
                                         dd[:, :cw])
                    nc.vector.tensor_add(h_all[:, c0:c1], hc_all[:, c0:c1],
                                         ee[:, :cw])
                    if s < nsweep - 1:
                        # incremental shift for the next sweep, overlapped
                        # with the remaining chunks' compute:
                        # hprev[p, t, :] <- h_all[p-1, t, :] within the tile,
                        # the p=0 row from partition 127 of tile t-1, and
                        # the next chunk's first p=0 row (tile 0 row 0 stays
                        # frozen at zero).
                        hp = hprev_c[ci]
                        nc.sync.dma_start(out=hp[1:P, :, 0:16],
                                          in_=h_all[0:P - 1, c0:c1, :])
                        if cw > 1:
                            nc.sync.dma_start(
                                out=hp[0:1, 1:cw, 0:16],
                                in_=h_all[P - 1:P, c0:c1 - 1, :])
                        if ci + 1 < len(chunks):
                            nc.sync.dma_start(
                                out=hprev_c[ci + 1][0:1, 0:1, 0:16],
                                in_=h_all[P - 1:P, c1 - 1:c1, :])

            # ---------------- output ----------------
            nc.sync.dma_start(
                out=ho_d.ap().rearrange("(a p) n -> p a n", p=P),
                in_=h_all[:])


def _pad_slice(a, lo, hi):
    """a[lo:hi] with zero-padding for lo < 0."""
    if lo >= 0:
        return np.ascontiguousarray(a[lo:hi])
    pad = np.zeros((-lo,) + a.shape[1:], a.dtype)
    return np.ascontiguousarray(np.concatenate([pad, a[0:hi]], axis=0))


def pack_small(wh, wx, x, bx, bh, gb):
    """Concatenate the small per-timestep inputs into one [T, 199] fp16
    array (layout consumed by the kernel's pk tensor)."""
    n = wh.shape[0]
    return np.ascontiguousarray(np.concatenate([
        wh.reshape(n, -1), wx.reshape(n, -1), x.reshape(n, -1),
        bx.reshape(n, -1), bh.reshape(n, -1), gb.reshape(n, -1)],
        axis=1).astype(np.float16))


def kernel(inputs, a_list, gcn_wx, gcn_bx, gcn_wh, gcn_bh, gru_k, gru_b):
    inputs = np.ascontiguousarray(np.asarray(inputs, np.float32))
    a_list = np.ascontiguousarray(np.asarray(a_list, np.float32))
    gcn_wx = np.ascontiguousarray(np.asarray(gcn_wx, np.float32))
    gcn_bx = np.ascontiguousarray(np.asarray(gcn_bx, np.float32))
    gcn_wh = np.ascontiguousarray(np.asarray(gcn_wh, np.float32))
    gcn_bh = np.ascontiguousarray(np.asarray(gcn_bh, np.float32))
    gru_k = np.ascontiguousarray(np.asarray(gru_k, np.float16))
    gru_b = np.ascontiguousarray(np.asarray(gru_b, np.float32))

    nc = _build(NTILES, NSWEEP)
    if not nc.is_finalized():
        nc.finalize()

    in_maps = []
    for c in range(NCORES):
        lo = c * PER_CORE - MARGIN
        hi = c * PER_CORE + PER_CORE
        in_maps.append({
            "pk": pack_small(_pad_slice(gcn_wh, lo, hi)[:, 0, :],
                             _pad_slice(gcn_wx, lo, hi),
                             _pad_slice(inputs, lo, hi),
                             _pad_slice(gcn_bx, lo, hi),
                             _pad_slice(gcn_bh, lo, hi),
                             _pad_slice(gru_b, lo, hi)),
            "gk": _pad_slice(gru_k, lo, hi),
            "alist": a_list,
        })

    res = run_bass_kernel_spmd(nc, in_maps, core_ids=list(range(NCORES)))
    global LAST_RESULTS
    LAST_RESULTS = res
    out = np.concatenate(
        [res.results[c]["hout"][MARGIN:] for c in range(NCORES)], axis=0)
    return out.astype(np.float32)


LAST_RESULTS = None

